# revision 17
# baseline (speedup 1.0000x reference)
"""Trainium2 Bass kernel for CrossTrans block (dense_transformer).

Computation (per batch b):
  x   = xx[:, 288:384]                      # query stream  [96, N]
  q   = Wq'@x + qb                          # [96, N]   (6 heads x 16)
  k   = Wk'@xx + kb                         # [96, N]
  v   = Wv'@xx + vb                         # [192, N]  (6 heads x 32)
  attn= softmax(q_h^T k_h)  per head        # [N, N]
  av  = v_h @ attn^T                        # [32, N]
  out = relu(av)/denominator  -> Wp' -> +x -> Wo' -> relu
All BN scales folded into weights on host; p_bias folded into o_bias.

Sharding: 8 cores = 4 batches x 2 query-halves. Each core computes full
k/v for its batch (duplicated across the half-pair) and attention for its
1152 query pixels. No collectives.

Layout on device: scores kept transposed [keys(m) on partitions, queries(n)
free] so the AV matmul consumes exp(scores) directly without transposing
the big attention matrix. Softmax denominators via ones-matmul (col-tiled).
exp skips max-subtraction: |logit| <~ 70 stays within fp32/bf16 range.
"""

import numpy as np

NUM_HEADS = 6
KD = 16
DH = 32
B, C, Himg, Wimg = 4, 384, 48, 48
N = Himg * Wimg          # 2304
NH = N // 2              # 1152 queries per core
DIM_S = C // 4           # 96
NHKD = NUM_HEADS * KD    # 96
DHALL = NUM_HEADS * DH   # 192
NCORES = 8

NCH = 384                # query chunk (free dim of score matmuls)
NNC = NH // NCH          # 3 query chunks per core
MCH = 128                # key chunk (partition tile)
NMCH = N // MCH          # 18 key chunks
MG = 3                   # key chunks per exp group (3 psum banks)
NG = NMCH // MG          # 6 groups
KT = C // 128            # 3 contraction tiles over channels


def build_nc():
    import concourse.bacc as bacc
    import concourse.mybir as mybir
    from concourse.tile import TileContext

    fp32 = mybir.dt.float32
    bf16 = mybir.dt.bfloat16
    AF = mybir.ActivationFunctionType
    OP = mybir.AluOpType

    nc = bacc.Bacc("TRN2", target_bir_lowering=False)

    xx_d = nc.dram_tensor("xx", [C, N], fp32, kind="ExternalInput")
    xh_d = nc.dram_tensor("xh", [DIM_S, NH], fp32, kind="ExternalInput")
    wk_d = nc.dram_tensor("wkrep", [C, 6 * NHKD], fp32, kind="ExternalInput")
    wv_d = nc.dram_tensor("wvT", [C, DHALL], fp32, kind="ExternalInput")
    wq_d = nc.dram_tensor("wqrep", [DIM_S, 6 * NHKD], fp32, kind="ExternalInput")
    wp_d = nc.dram_tensor("wpT", [DHALL, DIM_S], fp32, kind="ExternalInput")
    wo_d = nc.dram_tensor("woT", [DIM_S, C], fp32, kind="ExternalInput")
    kb_d = nc.dram_tensor("kbrep", [NHKD, 6], fp32, kind="ExternalInput")
    qb_d = nc.dram_tensor("qbrep", [NHKD, 6], fp32, kind="ExternalInput")
    vbb_d = nc.dram_tensor("vbb", [128, DHALL], fp32, kind="ExternalInput")
    ob_d = nc.dram_tensor("ob", [128, 3], fp32, kind="ExternalInput")
    y_d = nc.dram_tensor("y", [C, NH], fp32, kind="ExternalOutput")

    xx_t = xx_d[:, :].rearrange("(t p) n -> t p n", p=128)   # [3,128,N]
    wk_t = wk_d[:, :].rearrange("(t p) m -> t p m", p=128)   # [3,128,576]
    wv_t = wv_d[:, :].rearrange("(t p) m -> t p m", p=128)   # [3,128,192]

    with TileContext(nc) as tc:
        with tc.tile_pool(name="persist", bufs=1) as pp:
            # ---- load weights / biases ----
            wq_sb = pp.tile([DIM_S, 6 * NHKD], fp32, tag="wq")
            nc.sync.dma_start(out=wq_sb, in_=wq_d[:, :])
            wp0_sb = pp.tile([128, DIM_S], fp32, tag="wp0")
            nc.sync.dma_start(out=wp0_sb, in_=wp_d[0:128, :])
            wp1_sb = pp.tile([64, DIM_S], fp32, tag="wp1")
            nc.sync.dma_start(out=wp1_sb, in_=wp_d[128:192, :])
            wo_sb = pp.tile([DIM_S, C], fp32, tag="wo")
            nc.sync.dma_start(out=wo_sb, in_=wo_d[:, :])
            kb_sb = pp.tile([NHKD, 6], fp32, tag="kb")
            nc.sync.dma_start(out=kb_sb, in_=kb_d[:, :])
            qb_sb = pp.tile([NHKD, 6], fp32, tag="qb")
            nc.sync.dma_start(out=qb_sb, in_=qb_d[:, :])
            vbb_sb = pp.tile([128, DHALL], fp32, tag="vbb")
            nc.sync.dma_start(out=vbb_sb, in_=vbb_d[:, :])
            ob_sb = pp.tile([128, 3], fp32, tag="ob")
            nc.sync.dma_start(out=ob_sb, in_=ob_d[:, :])
            ones_sb = pp.tile([128, 32], bf16, tag="ones")
            nc.vector.memset(ones_sb, 1.0)
            wk_sb = [pp.tile([128, 6 * NHKD], fp32, tag=f"wk{t}", name=f"wk{t}")
                     for t in range(KT)]
            wv_sb = [pp.tile([128, DHALL], fp32, tag=f"wv{t}", name=f"wv{t}")
                     for t in range(KT)]
            for t in range(KT):
                nc.sync.dma_start(out=wk_sb[t], in_=wk_t[t])
                nc.sync.dma_start(out=wv_sb[t], in_=wv_t[t])

            xh_sb = pp.tile([DIM_S, NH], fp32, tag="xh")
            nc.sync.dma_start(out=xh_sb, in_=xh_d[:, :])

            # persistent conv outputs (pre-replicated at 3 row strips)
            qrep = [pp.tile([NHKD, NH], fp32, tag=f"qrep{h}", name=f"qrep{h}")
                    for h in range(NUM_HEADS)]
            vT = [pp.tile([128, DHALL], bf16, tag=f"vt{c}", name=f"vt{c}")
                  for c in range(NMCH)]
            # k replicated at 3 row-strips per head for row-tiled score MMs
            krep = [pp.tile([96, N], fp32, tag=f"krep{h}", name=f"krep{h}")
                    for h in range(NUM_HEADS)]
            # outputs of attention (normalized, relu'd), input to Wp conv
            rhs_p0 = pp.tile([128, NH], fp32, tag="rhs0")   # heads 0-3
            rhs_p1 = pp.tile([64, NH], fp32, tag="rhs1")    # heads 4-5
            xres_sb = pp.tile([DIM_S, NH], fp32, tag="xres")

            # ---- conv phase ----
            with tc.tile_pool(name="convin", bufs=1) as cip, \
                 tc.tile_pool(name="convps", bufs=2, space="PSUM") as cps:
                xx_sb = [cip.tile([128, N], fp32, tag=f"xx{t}", name=f"xx{t}")
                         for t in range(KT)]
                for t in range(KT):
                    nc.sync.dma_start(out=xx_sb[t], in_=xx_t[t])

                # k per head, pre-replicated at 3 strips: [96, N]
                for h in range(NUM_HEADS):
                    for j in range(N // NCH):
                        psk = cps.tile([NHKD, NCH], fp32, tag="psk")
                        for t in range(KT):
                            nc.tensor.matmul(
                                out=psk,
                                lhsT=wk_sb[t][:, NHKD * h:NHKD * (h + 1)],
                                rhs=xx_sb[t][:, j * NCH:(j + 1) * NCH],
                                start=(t == 0), stop=(t == KT - 1))
                        nc.vector.tensor_scalar(
                            out=krep[h][:, j * NCH:(j + 1) * NCH], in0=psk,
                            scalar1=kb_sb[:, h:h + 1], scalar2=None,
                            op0=OP.add)

                # q per head, pre-replicated: [96, NH]
                for h in range(NUM_HEADS):
                    for j in range(NNC):
                        psq = cps.tile([NHKD, NCH], fp32, tag="psq")
                        nc.tensor.matmul(
                            out=psq,
                            lhsT=wq_sb[:, NHKD * h:NHKD * (h + 1)],
                            rhs=xh_sb[:, j * NCH:(j + 1) * NCH],
                            start=True, stop=True)
                        nc.vector.tensor_scalar(
                            out=qrep[h][:, j * NCH:(j + 1) * NCH], in0=psq,
                            scalar1=qb_sb[:, h:h + 1], scalar2=None,
                            op0=OP.add)

                # vT chunks: vT[m,d] = xx_chunk^T @ Wv'^T  (+ vb broadcast)
                for c in range(NMCH):
                    psv = cps.tile([128, DHALL], fp32, tag="psv")
                    for t in range(KT):
                        nc.tensor.matmul(
                            out=psv,
                            lhsT=xx_sb[t][:, c * MCH:(c + 1) * MCH],
                            rhs=wv_sb[t],
                            start=(t == 0), stop=(t == KT - 1))
                    nc.vector.tensor_tensor(
                        out=vT[c], in0=psv, in1=vbb_sb, op=OP.add)

            # ---- attention ----
            with tc.tile_pool(name="attn_sb", bufs=1) as asb, \
                 tc.tile_pool(name="attn_ps", bufs=1, space="PSUM") as aps:
                for j in range(NNC):           # query chunk
                    nsl = slice(j * NCH, (j + 1) * NCH)
                    e_t = {}
                    for h in range(NUM_HEADS):
                        e_t[h] = asb.tile([128, NMCH * NCH], bf16,
                                          tag="e", bufs=4, name=f"e{h}")
                        for g in range(NG):    # 3 key-chunks per group
                            pss = aps.tile([128, 3 * 512], fp32,
                                           tag="pss", bufs=2)
                            for s in range(MG):
                                c = MG * g + s
                                nc.tensor.matmul(
                                    out=pss[:, 512 * s:512 * s + NCH],
                                    lhsT=krep[h][32 * s:32 * s + KD,
                                                 c * MCH:(c + 1) * MCH],
                                    rhs=qrep[h][32 * s:32 * s + KD, nsl],
                                    start=True, stop=True)
                            src = pss.rearrange("p (b n) -> p b n", n=512)
                            dst = e_t[h][:, g * MG * NCH:(g + 1) * MG * NCH]
                            nc.scalar.activation(
                                out=dst.rearrange("p (b n) -> p b n", n=NCH),
                                in_=src[:, :, 0:NCH], func=AF.Exp)

                    # AV + denominator, col-tiled
                    for pi, heads in enumerate(((0, 1, 2, 3), (4, 5))):
                        npart = 32 * len(heads)
                        av = aps.tile([128, NCH], fp32, tag="av")
                        dn = aps.tile([128, NCH], fp32, tag="dn")
                        for c in range(NMCH):
                            st, sp = (c == 0), (c == NMCH - 1)
                            for i, h in enumerate(heads):
                                nc.tensor.matmul(
                                    out=av[32 * i:32 * i + 32, :],
                                    lhsT=vT[c][:, 32 * h:32 * h + 32],
                                    rhs=e_t[h][:, c * NCH:(c + 1) * NCH],
                                    start=st, stop=sp, skip_group_check=True,
                                    tile_position=(0, 32 * i))
                        for c in range(NMCH):
                            st, sp = (c == 0), (c == NMCH - 1)
                            for i, h in enumerate(heads):
                                nc.tensor.matmul(
                                    out=dn[32 * i:32 * i + 32, :],
                                    lhsT=ones_sb[:, 0:32],
                                    rhs=e_t[h][:, c * NCH:(c + 1) * NCH],
                                    start=st, stop=sp, skip_group_check=True,
                                    tile_position=(0, 32 * i))
                        recip = asb.tile([128, NCH], fp32, tag="recip", bufs=2)
                        nc.vector.reciprocal_approx_fast(
                            out=recip[:npart], in_=dn[:npart])
                        dst = rhs_p0 if pi == 0 else rhs_p1
                        nc.vector.scalar_tensor_tensor(
                            out=dst[:npart, nsl], in0=av[:npart], scalar=0.0,
                            in1=recip[:npart], op0=OP.max, op1=OP.mult)

            # ---- Wp -> +x -> Wo -> relu -> DMA out ----
            with tc.tile_pool(name="out_sb", bufs=3) as osb, \
                 tc.tile_pool(name="out_ps", bufs=2, space="PSUM") as ops:
                for j in range(NNC):
                    nsl = slice(j * NCH, (j + 1) * NCH)
                    psp = ops.tile([DIM_S, NCH], fp32, tag="psp")
                    nc.tensor.matmul(out=psp, lhsT=wp0_sb,
                                     rhs=rhs_p0[:, nsl], start=True, stop=False)
                    nc.tensor.matmul(out=psp, lhsT=wp1_sb,
                                     rhs=rhs_p1[:, nsl], start=False, stop=True)
                    nc.vector.tensor_tensor(
                        out=xres_sb[:, nsl], in0=psp, in1=xh_sb[:, nsl],
                        op=OP.add)
                    for g in range(3):
                        psy = ops.tile([128, NCH], fp32, tag="psy", bufs=3)
                        nc.tensor.matmul(
                            out=psy, lhsT=wo_sb[:, 128 * g:128 * (g + 1)],
                            rhs=xres_sb[:, nsl], start=True, stop=True)
                        ysb = osb.tile([128, NCH], fp32, tag="ysb")
                        nc.vector.tensor_scalar(
                            out=ysb, in0=psy, scalar1=ob_sb[:, g:g + 1],
                            scalar2=0.0, op0=OP.add, op1=OP.max)
                        nc.sync.dma_start(
                            out=y_d[128 * g:128 * (g + 1), nsl], in_=ysb)
    return nc


def kernel(**inputs):
    import os
    from concourse.bass_utils import run_bass_kernel_spmd

    xx = np.asarray(inputs["xx"], dtype=np.float32)
    Wq = np.asarray(inputs["Wq"], dtype=np.float32)
    Wk = np.asarray(inputs["Wk"], dtype=np.float32)
    Wv = np.asarray(inputs["Wv"], dtype=np.float32)
    Wp = np.asarray(inputs["Wp"], dtype=np.float32)
    Wo = np.asarray(inputs["Wo"], dtype=np.float32)

    wqT = np.ascontiguousarray((inputs["q_scale"][:, None] * Wq).T)
    wkT = np.ascontiguousarray((inputs["k_scale"][:, None] * Wk).T)
    wvT = np.ascontiguousarray((inputs["v_scale"][:, None] * Wv).T)

    # column-replicated k/q weights: head h occupies cols [96h, 96h+96),
    # with its 16 kd rows copied at 3 row strips (32s + kd), zeros elsewhere.
    wkrep = np.zeros((C, 6 * NHKD), np.float32)
    wqrep = np.zeros((DIM_S, 6 * NHKD), np.float32)
    kbrep = np.zeros((NHKD, 6), np.float32)
    qbrep = np.zeros((NHKD, 6), np.float32)
    for h in range(NUM_HEADS):
        for s in range(3):
            cols = slice(NHKD * h + 32 * s, NHKD * h + 32 * s + KD)
            wkrep[:, cols] = wkT[:, KD * h:KD * (h + 1)]
            wqrep[:, cols] = wqT[:, KD * h:KD * (h + 1)]
            kbrep[32 * s:32 * s + KD, h] = inputs["k_bias"][KD * h:KD * (h + 1)]
            qbrep[32 * s:32 * s + KD, h] = inputs["q_bias"][KD * h:KD * (h + 1)]
    wpT = np.ascontiguousarray((inputs["p_scale"][:, None] * Wp).T)
    Wo2 = inputs["o_scale"][:, None] * Wo
    woT = np.ascontiguousarray(Wo2.T)
    ob2 = inputs["o_bias"] + Wo2 @ inputs["p_bias"]
    ob = np.ascontiguousarray(ob2.reshape(3, 128).T)   # [128, 3]
    vbb = np.ascontiguousarray(np.tile(inputs["v_bias"][None, :], (128, 1)))

    xx_flat = xx.reshape(B, C, N)
    shared = dict(wkrep=wkrep, wvT=wvT, wqrep=wqrep, wpT=wpT, woT=woT,
                  kbrep=kbrep, qbrep=qbrep, vbb=vbb, ob=ob)
    shared = {k: v.astype(np.float32) for k, v in shared.items()}

    in_maps = []
    for core in range(NCORES):
        b, half = core // 2, core % 2
        xxb = np.ascontiguousarray(xx_flat[b])
        xh = np.ascontiguousarray(
            xx_flat[b][3 * DIM_S:, half * NH:(half + 1) * NH])
        in_maps.append(dict(xx=xxb, xh=xh, **shared))

    nc = build_nc()
    if not nc.is_finalized():
        nc.finalize()
    trace = bool(int(os.environ.get("KERNEL_TRACE", "0")))
    res = run_bass_kernel_spmd(nc, in_maps, list(range(NCORES)),
                               trace=trace)
    if trace:
        kernel.last_result = res

    out = np.empty((B, C, N), dtype=np.float32)
    for core in range(NCORES):
        b, half = core // 2, core % 2
        out[b][:, half * NH:(half + 1) * NH] = res.results[core]["y"]
    return out.reshape(B, C, Himg, Wimg)


# revision 19
# speedup vs baseline: 1.6030x; 1.6030x over previous
"""Trainium2 Bass kernel for CrossTrans block (dense_transformer).

Computation (per batch b):
  x   = xx[:, 288:384]                      # query stream  [96, N]
  q   = Wq'@x + qb                          # [96, N]   (6 heads x 16)
  k   = Wk'@xx + kb                         # [96, N]
  v   = Wv'@xx + vb                         # [192, N]  (6 heads x 32)
  attn= softmax(q_h^T k_h)  per head        # [N, N]
  av  = v_h @ attn^T                        # [32, N]
  out = relu(av)/denominator  -> Wp' -> +x -> Wo' -> relu
All BN scales folded into weights on host; p_bias folded into o_bias.

Sharding: 8 cores = 4 batches x 2 query-halves. Each core computes full
k/v for its batch (duplicated across the half-pair) and attention for its
1152 query pixels. No collectives.

Layout on device: scores kept transposed [keys(m) on partitions, queries(n)
free] so the AV matmul consumes exp(scores) directly without transposing
the big attention matrix. Softmax denominators via ones-matmul (col-tiled).
exp skips max-subtraction: |logit| <~ 70 stays within fp32/bf16 range.
"""

import numpy as np

NUM_HEADS = 6
KD = 16
DH = 32
B, C, Himg, Wimg = 4, 384, 48, 48
N = Himg * Wimg          # 2304
NH = N // 2              # 1152 queries per core
DIM_S = C // 4           # 96
NHKD = NUM_HEADS * KD    # 96
DHALL = NUM_HEADS * DH   # 192
NCORES = 8

NCH = 384                # query chunk (free dim of score matmuls)
NNC = NH // NCH          # 3 query chunks per core
MCH = 128                # key chunk (partition tile)
NMCH = N // MCH          # 18 key chunks
MG = 3                   # key chunks per exp group (3 psum banks)
NG = NMCH // MG          # 6 groups
KT = C // 128            # 3 contraction tiles over channels


def build_nc():
    import concourse.bacc as bacc
    import concourse.mybir as mybir
    from concourse.tile import TileContext

    fp32 = mybir.dt.float32
    bf16 = mybir.dt.bfloat16
    AF = mybir.ActivationFunctionType
    OP = mybir.AluOpType

    nc = bacc.Bacc("TRN2", target_bir_lowering=False)

    xx_d = nc.dram_tensor("xx", [C, N], fp32, kind="ExternalInput")
    xh_d = nc.dram_tensor("xh", [DIM_S, NH], fp32, kind="ExternalInput")
    wk_d = nc.dram_tensor("wkT", [C, NHKD], bf16, kind="ExternalInput")
    wv_d = nc.dram_tensor("wvT", [C, DHALL], bf16, kind="ExternalInput")
    wq_d = nc.dram_tensor("wqT", [DIM_S, NHKD], bf16, kind="ExternalInput")
    wp_d = nc.dram_tensor("wpT", [DHALL, DIM_S], fp32, kind="ExternalInput")
    wo_d = nc.dram_tensor("woT", [DIM_S, C], fp32, kind="ExternalInput")
    kb_d = nc.dram_tensor("kb", [NHKD, 1], fp32, kind="ExternalInput")
    qb_d = nc.dram_tensor("qb", [NHKD, 1], fp32, kind="ExternalInput")
    vbb_d = nc.dram_tensor("vbb", [128, DHALL], fp32, kind="ExternalInput")
    ob_d = nc.dram_tensor("ob", [128, 3], fp32, kind="ExternalInput")
    y_d = nc.dram_tensor("y", [C, NH], fp32, kind="ExternalOutput")

    xx_t = xx_d[:, :].rearrange("(t p) n -> t p n", p=128)   # [3,128,N]
    wk_t = wk_d[:, :].rearrange("(t p) m -> t p m", p=128)   # [3,128,576]
    wv_t = wv_d[:, :].rearrange("(t p) m -> t p m", p=128)   # [3,128,192]

    with TileContext(nc) as tc:
        with tc.tile_pool(name="persist", bufs=1) as pp:
            # ---- load weights / biases ----
            wq_sb = pp.tile([DIM_S, NHKD], bf16, tag="wq")
            nc.sync.dma_start(out=wq_sb, in_=wq_d[:, :])
            wp0_sb = pp.tile([128, DIM_S], fp32, tag="wp0")
            nc.sync.dma_start(out=wp0_sb, in_=wp_d[0:128, :])
            wp1_sb = pp.tile([64, DIM_S], fp32, tag="wp1")
            nc.sync.dma_start(out=wp1_sb, in_=wp_d[128:192, :])
            wo_sb = pp.tile([DIM_S, C], fp32, tag="wo")
            nc.sync.dma_start(out=wo_sb, in_=wo_d[:, :])
            kb_sb = pp.tile([NHKD, 1], fp32, tag="kb")
            nc.sync.dma_start(out=kb_sb, in_=kb_d[:, :])
            qb_sb = pp.tile([NHKD, 1], fp32, tag="qb")
            nc.sync.dma_start(out=qb_sb, in_=qb_d[:, :])
            vbb_sb = pp.tile([128, DHALL], fp32, tag="vbb")
            nc.sync.dma_start(out=vbb_sb, in_=vbb_d[:, :])
            ob_sb = pp.tile([128, 3], fp32, tag="ob")
            nc.sync.dma_start(out=ob_sb, in_=ob_d[:, :])
            ones_sb = pp.tile([128, 32], bf16, tag="ones")
            nc.vector.memset(ones_sb, 1.0)
            wk_sb = [pp.tile([128, NHKD], bf16, tag=f"wk{t}", name=f"wk{t}")
                     for t in range(KT)]
            wv_sb = [pp.tile([128, DHALL], bf16, tag=f"wv{t}", name=f"wv{t}")
                     for t in range(KT)]
            for t in range(KT):
                nc.sync.dma_start(out=wk_sb[t], in_=wk_t[t])
                nc.sync.dma_start(out=wv_sb[t], in_=wv_t[t])

            xh_sb = pp.tile([DIM_S, NH], fp32, tag="xh")
            nc.sync.dma_start(out=xh_sb, in_=xh_d[:, :])

            # persistent conv outputs, replicated at 3 row strips (bf16)
            qrep = [pp.tile([NHKD, NH], bf16, tag=f"qrep{h}", name=f"qrep{h}")
                    for h in range(NUM_HEADS)]
            xh_bf = pp.tile([DIM_S, NH], bf16, tag="xhbf")
            nc.gpsimd.dma_start(out=xh_bf, in_=xh_d[:, :])
            vT = [pp.tile([128, DHALL], bf16, tag=f"vt{c}", name=f"vt{c}")
                  for c in range(NMCH)]
            # k replicated at 3 row-strips per head for row-tiled score MMs
            krep = [pp.tile([96, N], bf16, tag=f"krep{h}", name=f"krep{h}")
                    for h in range(NUM_HEADS)]
            # outputs of attention (normalized, relu'd), input to Wp conv
            rhs_p0 = pp.tile([128, NH], fp32, tag="rhs0")   # heads 0-3
            rhs_p1 = pp.tile([64, NH], fp32, tag="rhs1")    # heads 4-5
            xres_sb = pp.tile([DIM_S, NH], fp32, tag="xres")

            # ---- conv phase ----
            with tc.tile_pool(name="convin", bufs=1) as cip, \
                 tc.tile_pool(name="convps", bufs=2, space="PSUM") as cps:
                xx_sb = [cip.tile([128, N], bf16, tag=f"xx{t}", name=f"xx{t}")
                         for t in range(KT)]
                for t in range(KT):
                    nc.gpsimd.dma_start(out=xx_sb[t], in_=xx_t[t])

                # dense k = WkT.T @ xx + kb  -> [96, N] bf16 staging
                k_st = cip.tile([NHKD, N], bf16, tag="k_st")
                for j in range(N // NCH):
                    psk = cps.tile([NHKD, NCH], fp32, tag="psk")
                    for t in range(KT):
                        nc.tensor.matmul(
                            out=psk, lhsT=wk_sb[t],
                            rhs=xx_sb[t][:, j * NCH:(j + 1) * NCH],
                            start=(t == 0), stop=(t == KT - 1))
                    nc.vector.tensor_scalar(
                        out=k_st[:, j * NCH:(j + 1) * NCH], in0=psk,
                        scalar1=kb_sb[:, 0:1], scalar2=None, op0=OP.add)

                # dense q = WqT.T @ xh + qb -> [96, NH] bf16 staging
                q_st = cip.tile([NHKD, NH], bf16, tag="q_st")
                for j in range(NNC):
                    psq = cps.tile([NHKD, NCH], fp32, tag="psq")
                    nc.tensor.matmul(
                        out=psq, lhsT=wq_sb,
                        rhs=xh_bf[:, j * NCH:(j + 1) * NCH],
                        start=True, stop=True)
                    nc.vector.tensor_scalar(
                        out=q_st[:, j * NCH:(j + 1) * NCH], in0=psq,
                        scalar1=qb_sb[:, 0:1], scalar2=None, op0=OP.add)

                # replicate each head's 16 kd rows at 3 row strips
                for h in range(NUM_HEADS):
                    for s in range(3):
                        nc.sync.dma_start(
                            out=krep[h][32 * s:32 * s + KD, :],
                            in_=k_st[KD * h:KD * (h + 1), :])
                        nc.sync.dma_start(
                            out=qrep[h][32 * s:32 * s + KD, :],
                            in_=q_st[KD * h:KD * (h + 1), :])

                # vT chunks: vT[m,d] = xx_chunk^T @ Wv'^T  (+ vb broadcast)
                for c in range(NMCH):
                    psv = cps.tile([128, DHALL], fp32, tag="psv")
                    for t in range(KT):
                        nc.tensor.matmul(
                            out=psv,
                            lhsT=xx_sb[t][:, c * MCH:(c + 1) * MCH],
                            rhs=wv_sb[t],
                            start=(t == 0), stop=(t == KT - 1))
                    nc.vector.tensor_tensor(
                        out=vT[c], in0=psv, in1=vbb_sb, op=OP.add)

            # ---- attention ----
            with tc.tile_pool(name="attn_sb", bufs=1) as asb, \
                 tc.tile_pool(name="attn_ps", bufs=1, space="PSUM") as aps:
                for j in range(NNC):           # query chunk
                    nsl = slice(j * NCH, (j + 1) * NCH)
                    e_t = {}
                    for h in range(NUM_HEADS):
                        e_t[h] = asb.tile([128, NMCH * NCH], bf16,
                                          tag="e", bufs=6, name=f"e{h}")
                        for g in range(NG):    # 3 key-chunks per group
                            pss = aps.tile([128, 3 * 512], fp32,
                                           tag="pss", bufs=2)
                            for s in range(MG):
                                c = MG * g + s
                                nc.tensor.matmul(
                                    out=pss[:, 512 * s:512 * s + NCH],
                                    lhsT=krep[h][32 * s:32 * s + KD,
                                                 c * MCH:(c + 1) * MCH],
                                    rhs=qrep[h][32 * s:32 * s + KD, nsl],
                                    start=True, stop=True)
                            src = pss.rearrange("p (b n) -> p b n", n=512)
                            dst = e_t[h][:, g * MG * NCH:(g + 1) * MG * NCH]
                            nc.scalar.activation(
                                out=dst.rearrange("p (b n) -> p b n", n=NCH),
                                in_=src[:, :, 0:NCH], func=AF.Exp)

                    # AV + denominator, col-tiled
                    for pi, heads in enumerate(((0, 1, 2, 3), (4, 5))):
                        npart = 32 * len(heads)
                        av = aps.tile([128, NCH], fp32, tag="av")
                        dn = aps.tile([128, NCH], fp32, tag="dn")
                        for c in range(NMCH):
                            st, sp = (c == 0), (c == NMCH - 1)
                            for i, h in enumerate(heads):
                                nc.tensor.matmul(
                                    out=av[32 * i:32 * i + 32, :],
                                    lhsT=vT[c][:, 32 * h:32 * h + 32],
                                    rhs=e_t[h][:, c * NCH:(c + 1) * NCH],
                                    start=st, stop=sp, skip_group_check=True,
                                    tile_position=(0, 32 * i))
                        for c in range(NMCH):
                            st, sp = (c == 0), (c == NMCH - 1)
                            for i, h in enumerate(heads):
                                nc.tensor.matmul(
                                    out=dn[32 * i:32 * i + 32, :],
                                    lhsT=ones_sb[:, 0:32],
                                    rhs=e_t[h][:, c * NCH:(c + 1) * NCH],
                                    start=st, stop=sp, skip_group_check=True,
                                    tile_position=(0, 32 * i))
                        recip = asb.tile([128, NCH], fp32, tag="recip", bufs=2)
                        nc.vector.reciprocal_approx_fast(
                            out=recip[:npart], in_=dn[:npart])
                        dst = rhs_p0 if pi == 0 else rhs_p1
                        nc.vector.scalar_tensor_tensor(
                            out=dst[:npart, nsl], in0=av[:npart], scalar=0.0,
                            in1=recip[:npart], op0=OP.max, op1=OP.mult)

            # ---- Wp -> +x -> Wo -> relu -> DMA out ----
            with tc.tile_pool(name="out_sb", bufs=3) as osb, \
                 tc.tile_pool(name="out_ps", bufs=2, space="PSUM") as ops:
                for j in range(NNC):
                    nsl = slice(j * NCH, (j + 1) * NCH)
                    psp = ops.tile([DIM_S, NCH], fp32, tag="psp")
                    nc.tensor.matmul(out=psp, lhsT=wp0_sb,
                                     rhs=rhs_p0[:, nsl], start=True, stop=False)
                    nc.tensor.matmul(out=psp, lhsT=wp1_sb,
                                     rhs=rhs_p1[:, nsl], start=False, stop=True)
                    nc.vector.tensor_tensor(
                        out=xres_sb[:, nsl], in0=psp, in1=xh_sb[:, nsl],
                        op=OP.add)
                    for g in range(3):
                        psy = ops.tile([128, NCH], fp32, tag="psy", bufs=3)
                        nc.tensor.matmul(
                            out=psy, lhsT=wo_sb[:, 128 * g:128 * (g + 1)],
                            rhs=xres_sb[:, nsl], start=True, stop=True)
                        ysb = osb.tile([128, NCH], fp32, tag="ysb")
                        nc.vector.tensor_scalar(
                            out=ysb, in0=psy, scalar1=ob_sb[:, g:g + 1],
                            scalar2=0.0, op0=OP.add, op1=OP.max)
                        nc.sync.dma_start(
                            out=y_d[128 * g:128 * (g + 1), nsl], in_=ysb)
    return nc


def kernel(**inputs):
    import os
    from concourse.bass_utils import run_bass_kernel_spmd

    xx = np.asarray(inputs["xx"], dtype=np.float32)
    Wq = np.asarray(inputs["Wq"], dtype=np.float32)
    Wk = np.asarray(inputs["Wk"], dtype=np.float32)
    Wv = np.asarray(inputs["Wv"], dtype=np.float32)
    Wp = np.asarray(inputs["Wp"], dtype=np.float32)
    Wo = np.asarray(inputs["Wo"], dtype=np.float32)

    from ml_dtypes import bfloat16

    wqT = np.ascontiguousarray((inputs["q_scale"][:, None] * Wq).T).astype(bfloat16)
    wkT = np.ascontiguousarray((inputs["k_scale"][:, None] * Wk).T).astype(bfloat16)
    wvT = np.ascontiguousarray((inputs["v_scale"][:, None] * Wv).T).astype(bfloat16)
    wpT = np.ascontiguousarray((inputs["p_scale"][:, None] * Wp).T)
    Wo2 = inputs["o_scale"][:, None] * Wo
    woT = np.ascontiguousarray(Wo2.T)
    ob2 = inputs["o_bias"] + Wo2 @ inputs["p_bias"]
    ob = np.ascontiguousarray(ob2.reshape(3, 128).T)   # [128, 3]
    vbb = np.ascontiguousarray(np.tile(inputs["v_bias"][None, :], (128, 1)))

    kb = np.ascontiguousarray(inputs["k_bias"][:, None]).astype(np.float32)
    qb = np.ascontiguousarray(inputs["q_bias"][:, None]).astype(np.float32)
    xx_flat = xx.reshape(B, C, N)
    shared = dict(wkT=wkT, wvT=wvT, wqT=wqT,
                  wpT=wpT.astype(np.float32), woT=woT.astype(np.float32),
                  kb=kb, qb=qb, vbb=vbb.astype(np.float32),
                  ob=ob.astype(np.float32))

    in_maps = []
    for core in range(NCORES):
        b, half = core // 2, core % 2
        xxb = np.ascontiguousarray(xx_flat[b])
        xh = np.ascontiguousarray(
            xx_flat[b][3 * DIM_S:, half * NH:(half + 1) * NH])
        in_maps.append(dict(xx=xxb, xh=xh, **shared))

    nc = build_nc()
    if not nc.is_finalized():
        nc.finalize()
    trace = bool(int(os.environ.get("KERNEL_TRACE", "0")))
    res = run_bass_kernel_spmd(nc, in_maps, list(range(NCORES)),
                               trace=trace)
    if trace:
        kernel.last_result = res

    out = np.empty((B, C, N), dtype=np.float32)
    for core in range(NCORES):
        b, half = core // 2, core % 2
        out[b][:, half * NH:(half + 1) * NH] = res.results[core]["y"]
    return out.reshape(B, C, Himg, Wimg)
